# revision 5
# baseline (speedup 1.0000x reference)
"""Trainium2 Bass kernel for CapsNet dynamic routing (nn_Capsule_13692355740297).

Math (per batch element):
    u_hat[i, (n,d)] = u[i, :] @ W[:, (n,d)]            # never materialized
    iter1: c uniform 1/10  -> s1 = 0.1 * (sum_i u_i) W  (c-independent => host)
    iter k: b[i, n] = u_i . P_n   with P_n = W_n o_n    # contract Din on PE
            c = softmax_n(b)                            # free-dim softmax, [i,n]
            R[n, :] = sum_i c[i, n] u_i                 # cc stationary, U moving
            s[n, :] = R[n, :] @ W_n                     # transpose + mask trick
            o = squash(s)
Sharding: data-parallel over batch, 8 batch elements per core, no collectives.

Host pre-processing (allowed: kernel() receives full f32 inputs):
  - UT fp8(e3m4) [d, i] per batch  -> b-logit stationary tiles (FWL fast loads)
  - U  bf16 tile-layout [p, (b,j,d)] -> R-matmul moving operand
  - V2 = gamma * (W_n @ o1_n) from host iter-1 (o1 = squash(0.1 * sum_i u_i @ W))
  - gamma folded into capsule masks so on-chip V3 is gamma-scaled too;
    softmax uses exp(b/gamma) via the activation scale.
"""

import numpy as np

B, I_FULL, DIN = 64, 4096, 128
NCAP, DCAP = 10, 16
KND = NCAP * DCAP  # 160
NCORES = 8
BC = B // NCORES  # 8 batch elements per core
NT = I_FULL // 128  # 32 i-tiles per batch
EPS = 1e-7
FP8 = True
GAMMA = 32.0 if FP8 else 1.0


def build_nc(bc=BC, nt=NT, fp8=FP8):
    import concourse.bacc as bacc
    import concourse.mybir as mybir
    from concourse.tile import TileContext

    fp32 = mybir.dt.float32
    bf16 = mybir.dt.bfloat16
    dtu = mybir.dt.float8e3 if fp8 else mybir.dt.bfloat16
    AX = mybir.AxisListType
    ALU = mybir.AluOpType
    ACTF = mybir.ActivationFunctionType

    il = nt * 128  # I per batch

    nc = bacc.Bacc(trn_type="TRN2")
    ut_h = nc.dram_tensor("ut", [bc, 128, il], dtu, kind="ExternalInput")
    u16_h = nc.dram_tensor("u16", [128, bc * il], bf16, kind="ExternalInput")
    v2_h = nc.dram_tensor("v2", [128, bc * NCAP], dtu, kind="ExternalInput")
    wbf_h = nc.dram_tensor("wbf", [128, KND], bf16, kind="ExternalInput")
    wt_hi_h = nc.dram_tensor("wt_hi", [128, DIN], bf16, kind="ExternalInput")
    wt_lo_h = nc.dram_tensor("wt_lo", [32, DIN], bf16, kind="ExternalInput")
    m_hi_h = nc.dram_tensor("m_hi", [128, NCAP], fp32, kind="ExternalInput")
    m_lo_h = nc.dram_tensor("m_lo", [32, NCAP], fp32, kind="ExternalInput")
    identf_h = nc.dram_tensor("identf", [32, 32], fp32, kind="ExternalInput")
    identb_h = nc.dram_tensor("identb", [32, 32], bf16, kind="ExternalInput")
    ones_h = nc.dram_tensor("ones", [128, 1], bf16, kind="ExternalInput")
    out_h = nc.dram_tensor("out", [bc, KND], fp32, kind="ExternalOutput")

    with TileContext(nc) as tc:
        with (
            tc.tile_pool(name="big", bufs=1) as big,
            tc.tile_pool(name="sb3", bufs=3) as sb3,
            tc.tile_pool(name="psB", bufs=2, space="PSUM") as psB,
            tc.tile_pool(name="psR", bufs=2, space="PSUM") as psR,
            tc.tile_pool(name="psS", bufs=2, space="PSUM") as psS,
            tc.tile_pool(name="psT", bufs=2, space="PSUM") as psT,
        ):
            # ---------- persistent SBUF ----------
            UT = big.tile([128, bc * il], dtu, name="UT_sb")   # [d, (b,i)]
            U16 = big.tile([128, bc * il], bf16, name="U16_sb")  # [p, (b,j,d)]
            V2 = big.tile([128, bc * NCAP], dtu, name="V2_sb")
            wbf = big.tile([128, KND], bf16, name="wbf_sb")
            wt_hi = big.tile([128, DIN], bf16, name="wt_hi_sb")
            wt_lo = big.tile([32, DIN], bf16, name="wt_lo_sb")
            m_hi = big.tile([128, NCAP], fp32, name="m_hi_sb")
            m_lo = big.tile([32, NCAP], fp32, name="m_lo_sb")
            identf = big.tile([32, 32], fp32, name="identf_sb")
            identb = big.tile([32, 32], bf16, name="identb_sb")
            onesb = big.tile([128, 1], bf16, name="ones_sb")

            nc.sync.dma_start(out=V2[:, :], in_=v2_h.ap())
            nc.sync.dma_start(out=wbf[:, :], in_=wbf_h.ap())
            nc.sync.dma_start(out=wt_hi[:, :], in_=wt_hi_h.ap())
            nc.sync.dma_start(out=wt_lo[:, :], in_=wt_lo_h.ap())
            nc.sync.dma_start(out=m_hi[:, :], in_=m_hi_h.ap())
            nc.sync.dma_start(out=m_lo[:, :], in_=m_lo_h.ap())
            nc.sync.dma_start(out=identf[:, :], in_=identf_h.ap())
            nc.sync.dma_start(out=identb[:, :], in_=identb_h.ap())
            nc.sync.dma_start(out=onesb[:, :], in_=ones_h.ap())

            UTv = UT[:, :].rearrange("p (b i) -> p b i", b=bc, i=il)
            U16v = U16[:, :].rearrange("p (b j d) -> p b j d", b=bc, j=nt, d=128)
            Wv = wbf[:, :].rearrange("p (n d) -> p n d", n=NCAP)

            # ---------- bulk load, batch-pipelined ----------
            for b in range(bc):
                nc.gpsimd.dma_start(out=UTv[:, b, :], in_=ut_h.ap()[b])
                half = il // 2
                nc.gpsimd.dma_start(
                    out=U16[:, b * il : b * il + half],
                    in_=u16_h.ap()[:, b * il : b * il + half],
                )
                nc.gpsimd.dma_start(
                    out=U16[:, b * il + half : (b + 1) * il],
                    in_=u16_h.ap()[:, b * il + half : (b + 1) * il],
                )

            def routing_iter(it, b, Vb):
                """One routing iteration for batch b. Vb: [128, NCAP] dtu tile/view.
                Returns ob [1, KND] f32 sbuf tile (squashed output)."""
                btp = psB.tile([128, nt * NCAP], fp32, name=f"btp{it}_{b}", tag="btp")
                for j in range(nt):
                    nc.tensor.matmul(
                        btp[:, NCAP * j : NCAP * (j + 1)],
                        UTv[:, b, 128 * j : 128 * (j + 1)],
                        Vb,
                    )
                eb = sb3.tile([128, nt * NCAP], fp32, name=f"eb{it}_{b}", tag="eb")
                nc.scalar.activation(eb[:, :], btp[:, :], ACTF.Exp, scale=1.0 / GAMMA)
                ebv = eb[:, :].rearrange("p (j n) -> p j n", j=nt)
                Z = sb3.tile([128, nt], fp32, name=f"Z{it}_{b}", tag="Z")
                nc.vector.reduce_sum(out=Z[:, :], in_=ebv, axis=AX.X, op=ALU.add)
                rZ = sb3.tile([128, nt], fp32, name=f"rZ{it}_{b}", tag="rZ")
                nc.vector.reciprocal(out=rZ[:, :], in_=Z[:, :])
                cc = sb3.tile([128, nt * NCAP], bf16, name=f"cc{it}_{b}", tag="cc")
                nc.vector.tensor_tensor(
                    out=cc[:, :].rearrange("p (j n) -> p j n", j=nt),
                    in0=ebv,
                    in1=rZ[:, :].unsqueeze(2).broadcast_to([128, nt, NCAP]),
                    op=ALU.mult,
                )
                # R^T accumulation: [10, 128] = sum_j cc_j^T @ U_j
                Rp = psR.tile([NCAP, 128], fp32, name=f"Rp{it}_{b}", tag="Rp")
                for j in range(nt):
                    nc.tensor.matmul(
                        Rp[:, :],
                        cc[:, NCAP * j : NCAP * (j + 1)],
                        U16v[:, b, j],
                        start=(j == 0),
                        stop=(j == nt - 1),
                    )
                Rs = sb3.tile([NCAP, 128], bf16, name=f"Rs{it}_{b}", tag="Rs")
                nc.scalar.copy(out=Rs[:, :], in_=Rp[:, :])
                Rt_p = psT.tile([128, NCAP], bf16, name=f"Rt{it}_{b}", tag="tp")
                nc.tensor.transpose(Rt_p[:, :], Rs[:, :], identb[:NCAP, :NCAP])
                prod = sb3.tile([128, KND], bf16, name=f"prod{it}_{b}", tag="prod")
                nc.vector.tensor_tensor(
                    out=prod[:, :].rearrange("p (n d) -> p n d", n=NCAP),
                    in0=Rt_p[:, :].unsqueeze(2).broadcast_to([128, NCAP, DCAP]),
                    in1=Wv,
                    op=ALU.mult,
                )
                sp = psS.tile([1, KND], fp32, name=f"sp{it}_{b}", tag="sp")
                nc.tensor.matmul(sp[:, :], onesb[:, :], prod[:, :])
                # squash on [1, KND]
                sq = sb3.tile([1, KND], fp32, name=f"sq{it}_{b}", tag="sq")
                nc.scalar.square(out=sq[:, :], in_=sp[:, :])
                q = sb3.tile([1, NCAP], fp32, name=f"q{it}_{b}", tag="q")
                nc.vector.reduce_sum(
                    out=q[:, :],
                    in_=sq[:, :].rearrange("p (n d) -> p n d", n=NCAP),
                    axis=AX.X, op=ALU.add,
                )
                nc.vector.tensor_scalar_add(q[:, :], q[:, :], EPS)
                rt = sb3.tile([1, NCAP], fp32, name=f"rt{it}_{b}", tag="rt")
                nc.scalar.sqrt(out=rt[:, :], in_=q[:, :])
                den = sb3.tile([1, NCAP], fp32, name=f"den{it}_{b}", tag="den")
                nc.vector.tensor_scalar_add(den[:, :], q[:, :], 1.0)
                rden = sb3.tile([1, NCAP], fp32, name=f"rden{it}_{b}", tag="rden")
                nc.vector.reciprocal(out=rden[:, :], in_=den[:, :])
                coef = sb3.tile([1, NCAP], fp32, name=f"coef{it}_{b}", tag="coef")
                nc.vector.tensor_tensor(
                    out=coef[:, :], in0=rt[:, :], in1=rden[:, :], op=ALU.mult
                )
                ob = sb3.tile([1, KND], fp32, name=f"ob{it}_{b}", tag="ob")
                nc.vector.tensor_tensor(
                    out=ob[:, :].rearrange("p (n d) -> p n d", n=NCAP),
                    in0=sp[:, :].rearrange("p (n d) -> p n d", n=NCAP),
                    in1=coef[:, :].unsqueeze(2).broadcast_to([1, NCAP, DCAP]),
                    op=ALU.mult,
                )
                return ob

            def build_V3(b, ob):
                """V3(b) = gamma * W_n @ o_n from ob [1, KND] f32; masks carry gamma."""
                oth_p = psT.tile([128, 1], fp32, name=f"oth{b}", tag="tp")
                otl_p = psT.tile([32, 1], fp32, name=f"otl{b}", tag="tp")
                nc.tensor.transpose(oth_p[:, :], ob[:, 0:128], identf[:1, :1])
                nc.tensor.transpose(otl_p[:, :], ob[:, 128:KND], identf[:1, :1])
                oeh = sb3.tile([128, NCAP], bf16, name=f"oeh{b}", tag="oeh")
                oel = sb3.tile([32, NCAP], bf16, name=f"oel{b}", tag="oel")
                nc.vector.tensor_tensor(
                    out=oeh[:, :],
                    in0=oth_p[:, :].broadcast_to([128, NCAP]),
                    in1=m_hi[:, :],
                    op=ALU.mult,
                )
                nc.vector.tensor_tensor(
                    out=oel[:, :],
                    in0=otl_p[:, :].broadcast_to([32, NCAP]),
                    in1=m_lo[:, :],
                    op=ALU.mult,
                )
                vp = psT.tile([128, NCAP], fp32, name=f"vp{b}", tag="tp")
                nc.tensor.matmul(vp[:, :], wt_hi[:, :], oeh[:, :], start=True, stop=False)
                nc.tensor.matmul(vp[:, :], wt_lo[:, :], oel[:, :], start=False, stop=True)
                V3 = sb3.tile([128, NCAP], dtu, name=f"V3_{b}", tag="V3")
                nc.scalar.copy(out=V3[:, :], in_=vp[:, :])
                return V3

            # ---------- skewed per-batch pipeline: iter2(b) || iter3(b-1) ----------
            pend = None  # (b, V3) awaiting iter 3
            for b in range(bc):
                ob2 = routing_iter(2, b, V2[:, NCAP * b : NCAP * (b + 1)])
                if pend is not None:
                    pb, V3b = pend
                    ob3 = routing_iter(3, pb, V3b[:, :])
                    nc.sync.dma_start(out=out_h.ap()[pb], in_=ob3[:, :])
                pend = (b, build_V3(b, ob2))
            pb, V3b = pend
            ob3 = routing_iter(3, pb, V3b[:, :])
            nc.sync.dma_start(out=out_h.ap()[pb], in_=ob3[:, :])

    nc.compile()
    return nc


def _squash_np(s):
    sq = (s * s).reshape(s.shape[0], NCAP, DCAP).sum(-1, keepdims=True) + EPS
    coef = np.sqrt(sq) / (1.0 + sq)
    return (coef * s.reshape(s.shape[0], NCAP, DCAP)).reshape(s.shape)


def make_in_maps(u_vecs, W, fp8=FP8):
    import ml_dtypes

    bf = ml_dtypes.bfloat16
    f8 = ml_dtypes.float8_e3m4 if fp8 else bf

    u = np.asarray(u_vecs, dtype=np.float32)
    W = np.asarray(W, dtype=np.float32)

    # host iter-1 (c uniform): o1 = squash(0.1 * (sum_i u_i) @ W), V2 = g*W_n@o1_n
    r0 = u.sum(axis=1)                      # [B, 128]
    o1 = _squash_np(0.1 * (r0 @ W))         # [B, 160]
    V2full = np.einsum(
        "dnk,bnk->bdn",
        W.reshape(DIN, NCAP, DCAP),
        o1.reshape(-1, NCAP, DCAP),
        optimize=True,
    )                                        # [B, 128, 10]

    mask = np.zeros((KND, NCAP), dtype=np.float32)
    for k in range(KND):
        mask[k, k // DCAP] = GAMMA
    WT = W.T.copy()
    consts = {
        "wbf": W.astype(bf),
        "wt_hi": WT[:128].astype(bf),
        "wt_lo": WT[128:].astype(bf),
        "m_hi": mask[:128],
        "m_lo": mask[128:],
        "identf": np.eye(32, dtype=np.float32),
        "identb": np.eye(32, dtype=np.float32).astype(bf),
        "ones": np.ones((128, 1), dtype=np.float32).astype(bf),
    }

    in_maps = []
    for c in range(NCORES):
        sl = u[c * BC : (c + 1) * BC]       # [8, 4096, 128]
        ut = np.ascontiguousarray(sl.transpose(0, 2, 1)).astype(f8)  # [8,128,4096]
        u16 = np.ascontiguousarray(
            sl.reshape(BC, NT, 128, 128).transpose(2, 0, 1, 3)
        ).reshape(128, BC * I_FULL).astype(bf)
        v2 = np.ascontiguousarray(
            V2full[c * BC : (c + 1) * BC].transpose(1, 0, 2)
        ).reshape(128, BC * NCAP)
        m = {"ut": ut, "u16": u16, "v2": (GAMMA * v2).astype(f8)}
        m.update(consts)
        in_maps.append(m)
    return in_maps


_CACHE = {}


def kernel(u_vecs, W):
    from concourse import bass_utils

    if "nc" not in _CACHE:
        _CACHE["nc"] = build_nc()
    nc = _CACHE["nc"]

    in_maps = make_in_maps(u_vecs, W)
    res = bass_utils.run_bass_kernel_spmd(nc, in_maps, core_ids=list(range(NCORES)))
    outs = [r["out"] for r in res.results]
    return np.concatenate(outs, axis=0).reshape(B, NCAP, DCAP).astype(np.float32)


# revision 7
# speedup vs baseline: 1.4715x; 1.4715x over previous
"""Trainium2 Bass kernel for CapsNet dynamic routing (nn_Capsule_13692355740297).

Math (per batch element):
    u_hat[i, (n,d)] = u[i, :] @ W[:, (n,d)]            # never materialized
    iter1: c uniform 1/10  -> s1 = 0.1 * (sum_i u_i) W  (c-independent => host)
    iter k: b[i, n] = u_i . P_n   with P_n = W_n o_n    # lhsT=UT tile (FWL)
            c = softmax_n(b)                            # free-dim softmax, [i,n]
            R^T[d, n] = sum_i u_i c[i, n]               # lhsT=U tile (FWL), rhs=cc
            s[n, :] = R_n @ W_n                         # mask-mult + ones matmul
            o = squash(s)                               # iter2 on-chip, iter3 host
Sharding: data-parallel over batch, 8 batch elements per core, no collectives.

Perf notes (HW-measured):
  - (128-col FWL LDWEIGHTS + 10-col MATMUL) pairs pipeline at ~27 ns;
    big moving streams serialize at ~128 ns/tile -> keep 10-col moving operands
    on both matmul flavors and feed data through the FWL weight path.
  - Scalar act-table reloads cost 1.5 us each; all scalar funcs kept inside the
    natural_log_exp_and_others set (exp/ln/square/copy): sqrt(q)=exp(0.5*ln(q)).
  - Per-batch slot pipeline: logits2(b) | logits3(b-1) | R2(b) | R3(b-1)
    keeps the PE busy while softmax/squash chains run on scalar/DVE.
"""

import numpy as np

B, I_FULL, DIN = 64, 4096, 128
NCAP, DCAP = 10, 16
KND = NCAP * DCAP  # 160
NCORES = 8
BC = B // NCORES  # 8 batch elements per core
NT = I_FULL // 128  # 32 i-tiles per batch
EPS = 1e-7
FP8 = True
GAMMA = 32.0 if FP8 else 1.0


def build_nc(bc=BC, nt=NT, fp8=FP8):
    import concourse.bacc as bacc
    import concourse.mybir as mybir
    from concourse.tile import TileContext

    fp32 = mybir.dt.float32
    bf16 = mybir.dt.bfloat16
    dtu = mybir.dt.float8e3 if fp8 else mybir.dt.bfloat16
    AX = mybir.AxisListType
    ALU = mybir.AluOpType
    ACTF = mybir.ActivationFunctionType

    il = nt * 128  # I per batch

    nc = bacc.Bacc(trn_type="TRN2")
    ut_h = nc.dram_tensor("ut", [bc, 128, il], dtu, kind="ExternalInput")
    u16_h = nc.dram_tensor("u16", [128, bc * il], bf16, kind="ExternalInput")
    v2_h = nc.dram_tensor("v2", [128, bc * NCAP], dtu, kind="ExternalInput")
    w32_h = nc.dram_tensor("w32", [128, KND], fp32, kind="ExternalInput")
    wt_hi_h = nc.dram_tensor("wt_hi", [128, DIN], bf16, kind="ExternalInput")
    wt_lo_h = nc.dram_tensor("wt_lo", [32, DIN], bf16, kind="ExternalInput")
    m_hi_h = nc.dram_tensor("m_hi", [128, NCAP], fp32, kind="ExternalInput")
    m_lo_h = nc.dram_tensor("m_lo", [32, NCAP], fp32, kind="ExternalInput")
    identf_h = nc.dram_tensor("identf", [32, 32], fp32, kind="ExternalInput")
    ones_h = nc.dram_tensor("ones", [128, 1], bf16, kind="ExternalInput")
    out_h = nc.dram_tensor("out", [bc, KND], fp32, kind="ExternalOutput")

    with TileContext(nc) as tc:
        with (
            tc.tile_pool(name="big", bufs=1) as big,
            tc.tile_pool(name="sb3", bufs=3) as sb3,
            tc.tile_pool(name="psB", bufs=2, space="PSUM") as psB,
            tc.tile_pool(name="psR", bufs=2, space="PSUM") as psR,
            tc.tile_pool(name="psS", bufs=2, space="PSUM") as psS,
            tc.tile_pool(name="psT", bufs=2, space="PSUM") as psT,
        ):
            # ---------- persistent SBUF ----------
            UT = big.tile([128, bc * il], dtu, name="UT_sb")   # [d, (b,i)]
            U16 = big.tile([128, bc * il], bf16, name="U16_sb")  # [p, (b,j,d)]
            V2 = big.tile([128, bc * NCAP], dtu, name="V2_sb")
            w32 = big.tile([128, KND], fp32, name="w32_sb")
            wt_hi = big.tile([128, DIN], bf16, name="wt_hi_sb")
            wt_lo = big.tile([32, DIN], bf16, name="wt_lo_sb")
            m_hi = big.tile([128, NCAP], fp32, name="m_hi_sb")
            m_lo = big.tile([32, NCAP], fp32, name="m_lo_sb")
            identf = big.tile([32, 32], fp32, name="identf_sb")
            onesb = big.tile([128, 1], bf16, name="ones_sb")

            nc.sync.dma_start(out=V2[:, :], in_=v2_h.ap())
            nc.sync.dma_start(out=w32[:, :], in_=w32_h.ap())
            nc.sync.dma_start(out=wt_hi[:, :], in_=wt_hi_h.ap())
            nc.sync.dma_start(out=wt_lo[:, :], in_=wt_lo_h.ap())
            nc.sync.dma_start(out=m_hi[:, :], in_=m_hi_h.ap())
            nc.sync.dma_start(out=m_lo[:, :], in_=m_lo_h.ap())
            nc.sync.dma_start(out=identf[:, :], in_=identf_h.ap())
            nc.sync.dma_start(out=onesb[:, :], in_=ones_h.ap())

            UTv = UT[:, :].rearrange("p (b i) -> p b i", b=bc, i=il)
            U16v = U16[:, :].rearrange("p (b j d) -> p b j d", b=bc, j=nt, d=128)
            Wv = w32[:, :].rearrange("p (n d) -> p n d", n=NCAP)

            # ---------- bulk load, batch-pipelined ----------
            for b in range(bc):
                nc.gpsimd.dma_start(out=UTv[:, b, :], in_=ut_h.ap()[b])
                half = il // 2
                nc.gpsimd.dma_start(
                    out=U16[:, b * il : b * il + half],
                    in_=u16_h.ap()[:, b * il : b * il + half],
                )
                nc.gpsimd.dma_start(
                    out=U16[:, b * il + half : (b + 1) * il],
                    in_=u16_h.ap()[:, b * il + half : (b + 1) * il],
                )

            def logits_half(it, b, Vb):
                """b-logit matmuls + softmax for batch b. Returns cc [128, nt*NCAP] bf16."""
                btp = psB.tile([128, nt * NCAP], fp32, name=f"btp{it}_{b}", tag="btp")
                for j in range(nt):
                    nc.tensor.matmul(
                        btp[:, NCAP * j : NCAP * (j + 1)],
                        UTv[:, b, 128 * j : 128 * (j + 1)],
                        Vb,
                    )
                eb = sb3.tile([128, nt * NCAP], fp32, name=f"eb{it}_{b}", tag="eb")
                nc.scalar.activation(eb[:, :], btp[:, :], ACTF.Exp, scale=1.0 / GAMMA)
                ebv = eb[:, :].rearrange("p (j n) -> p j n", j=nt)
                Z = sb3.tile([128, nt], fp32, name=f"Z{it}_{b}", tag="Z")
                nc.vector.reduce_sum(out=Z[:, :], in_=ebv, axis=AX.X, op=ALU.add)
                rZ = sb3.tile([128, nt], fp32, name=f"rZ{it}_{b}", tag="rZ")
                nc.vector.reciprocal(out=rZ[:, :], in_=Z[:, :])
                cc = sb3.tile([128, nt * NCAP], bf16, name=f"cc{it}_{b}", tag="cc")
                nc.vector.tensor_tensor(
                    out=cc[:, :].rearrange("p (j n) -> p j n", j=nt),
                    in0=ebv,
                    in1=rZ[:, :].unsqueeze(2).broadcast_to([128, nt, NCAP]),
                    op=ALU.mult,
                )
                return cc

            def r_half(it, b, cc):
                """R^T = sum_j U_j @ cc_j -> [128 d, 10 n]; s = ones^T (R*W) [1, KND]."""
                Rp = psR.tile([128, NCAP], fp32, name=f"Rp{it}_{b}", tag="Rp")
                for j in range(nt):
                    nc.tensor.matmul(
                        Rp[:, :],
                        U16v[:, b, j],
                        cc[:, NCAP * j : NCAP * (j + 1)],
                        start=(j == 0),
                        stop=(j == nt - 1),
                    )
                prod = sb3.tile([128, KND], bf16, name=f"prod{it}_{b}", tag="prod")
                nc.vector.tensor_tensor(
                    out=prod[:, :].rearrange("p (n d) -> p n d", n=NCAP),
                    in0=Rp[:, :].unsqueeze(2).broadcast_to([128, NCAP, DCAP]),
                    in1=Wv,
                    op=ALU.mult,
                )
                sp = psS.tile([1, KND], fp32, name=f"sp{it}_{b}", tag="sp")
                nc.tensor.matmul(sp[:, :], onesb[:, :], prod[:, :])
                return sp

            def squash2(b, sp):
                """squash on [1, KND]; sqrt via exp(0.5*ln) to stay in one act set."""
                sq = sb3.tile([1, KND], fp32, name=f"sq{b}", tag="sq")
                nc.scalar.square(out=sq[:, :], in_=sp[:, :])
                q = sb3.tile([1, NCAP], fp32, name=f"q{b}", tag="q")
                nc.vector.reduce_sum(
                    out=q[:, :],
                    in_=sq[:, :].rearrange("p (n d) -> p n d", n=NCAP),
                    axis=AX.X, op=ALU.add,
                )
                qe = sb3.tile([1, NCAP], fp32, name=f"qe{b}", tag="qe")
                nc.vector.tensor_scalar_add(qe[:, :], q[:, :], EPS)
                lq = sb3.tile([1, NCAP], fp32, name=f"lq{b}", tag="lq")
                nc.scalar.activation(lq[:, :], qe[:, :], ACTF.Ln)
                rt = sb3.tile([1, NCAP], fp32, name=f"rt{b}", tag="rt")
                nc.scalar.activation(rt[:, :], lq[:, :], ACTF.Exp, scale=0.5)
                den = sb3.tile([1, NCAP], fp32, name=f"den{b}", tag="den")
                nc.vector.tensor_scalar_add(den[:, :], qe[:, :], 1.0)
                rden = sb3.tile([1, NCAP], fp32, name=f"rden{b}", tag="rden")
                nc.vector.reciprocal(out=rden[:, :], in_=den[:, :])
                coef = sb3.tile([1, NCAP], fp32, name=f"coef{b}", tag="coef")
                nc.vector.tensor_tensor(
                    out=coef[:, :], in0=rt[:, :], in1=rden[:, :], op=ALU.mult
                )
                ob = sb3.tile([1, KND], fp32, name=f"ob{b}", tag="ob")
                nc.vector.tensor_tensor(
                    out=ob[:, :].rearrange("p (n d) -> p n d", n=NCAP),
                    in0=sp[:, :].rearrange("p (n d) -> p n d", n=NCAP),
                    in1=coef[:, :].unsqueeze(2).broadcast_to([1, NCAP, DCAP]),
                    op=ALU.mult,
                )
                return ob

            def build_V3(b, ob):
                """V3(b) = gamma * W_n @ o_n from ob [1, KND] f32; masks carry gamma."""
                oth_p = psT.tile([128, 1], fp32, name=f"oth{b}", tag="tp")
                otl_p = psT.tile([32, 1], fp32, name=f"otl{b}", tag="tp")
                nc.tensor.transpose(oth_p[:, :], ob[:, 0:128], identf[:1, :1])
                nc.tensor.transpose(otl_p[:, :], ob[:, 128:KND], identf[:1, :1])
                oeh = sb3.tile([128, NCAP], bf16, name=f"oeh{b}", tag="oeh")
                oel = sb3.tile([32, NCAP], bf16, name=f"oel{b}", tag="oel")
                nc.vector.tensor_tensor(
                    out=oeh[:, :],
                    in0=oth_p[:, :].broadcast_to([128, NCAP]),
                    in1=m_hi[:, :],
                    op=ALU.mult,
                )
                nc.vector.tensor_tensor(
                    out=oel[:, :],
                    in0=otl_p[:, :].broadcast_to([32, NCAP]),
                    in1=m_lo[:, :],
                    op=ALU.mult,
                )
                vp = psT.tile([128, NCAP], fp32, name=f"vp{b}", tag="tp")
                nc.tensor.matmul(vp[:, :], wt_hi[:, :], oeh[:, :], start=True, stop=False)
                nc.tensor.matmul(vp[:, :], wt_lo[:, :], oel[:, :], start=False, stop=True)
                V3 = sb3.tile([128, NCAP], dtu, name=f"V3_{b}", tag="V3")
                nc.scalar.copy(out=V3[:, :], in_=vp[:, :])
                return V3

            def finish3(b, sp):
                o3 = sb3.tile([1, KND], fp32, name=f"o3_{b}", tag="o3")
                nc.vector.tensor_copy(out=o3[:, :], in_=sp[:, :])
                nc.sync.dma_start(out=out_h.ap()[b], in_=o3[:, :])

            # ---------- skewed per-batch pipeline ----------
            prev = None  # (b, V3, cc3 pending)
            for b in range(bc):
                cc2 = logits_half(2, b, V2[:, NCAP * b : NCAP * (b + 1)])
                if prev is not None:
                    pb, V3b = prev
                    cc3 = logits_half(3, pb, V3b[:, :])
                sp2 = r_half(2, b, cc2)
                if prev is not None:
                    finish3(pb, r_half(3, pb, cc3))
                ob2 = squash2(b, sp2)
                prev = (b, build_V3(b, ob2))
            pb, V3b = prev
            cc3 = logits_half(3, pb, V3b[:, :])
            finish3(pb, r_half(3, pb, cc3))

    nc.compile()
    return nc


def _squash_np(s):
    sq = (s * s).reshape(s.shape[0], NCAP, DCAP).sum(-1, keepdims=True) + EPS
    coef = np.sqrt(sq) / (1.0 + sq)
    return (coef * s.reshape(s.shape[0], NCAP, DCAP)).reshape(s.shape)


def make_in_maps(u_vecs, W, fp8=FP8):
    import ml_dtypes

    bf = ml_dtypes.bfloat16
    f8 = ml_dtypes.float8_e3m4 if fp8 else bf

    u = np.asarray(u_vecs, dtype=np.float32)
    W = np.asarray(W, dtype=np.float32)

    # host iter-1 (c uniform): o1 = squash(0.1 * (sum_i u_i) @ W), V2 = g*W_n@o1_n
    r0 = u.sum(axis=1)                      # [B, 128]
    o1 = _squash_np(0.1 * (r0 @ W))         # [B, 160]
    V2full = np.einsum(
        "dnk,bnk->bdn",
        W.reshape(DIN, NCAP, DCAP),
        o1.reshape(-1, NCAP, DCAP),
        optimize=True,
    )                                        # [B, 128, 10]

    mask = np.zeros((KND, NCAP), dtype=np.float32)
    for k in range(KND):
        mask[k, k // DCAP] = GAMMA
    WT = W.T.copy()
    consts = {
        "w32": W,
        "wt_hi": WT[:128].astype(bf),
        "wt_lo": WT[128:].astype(bf),
        "m_hi": mask[:128],
        "m_lo": mask[128:],
        "identf": np.eye(32, dtype=np.float32),
        "ones": np.ones((128, 1), dtype=np.float32).astype(bf),
    }

    in_maps = []
    for c in range(NCORES):
        sl = u[c * BC : (c + 1) * BC]       # [8, 4096, 128]
        ut = np.ascontiguousarray(sl.transpose(0, 2, 1)).astype(f8)  # [8,128,4096]
        u16 = np.ascontiguousarray(
            sl.reshape(BC, NT, 128, 128).transpose(2, 0, 1, 3)
        ).reshape(128, BC * I_FULL).astype(bf)
        v2 = np.ascontiguousarray(
            V2full[c * BC : (c + 1) * BC].transpose(1, 0, 2)
        ).reshape(128, BC * NCAP)
        m = {"ut": ut, "u16": u16, "v2": (GAMMA * v2).astype(f8)}
        m.update(consts)
        in_maps.append(m)
    return in_maps


_CACHE = {}


def kernel(u_vecs, W):
    from concourse import bass_utils

    if "nc" not in _CACHE:
        _CACHE["nc"] = build_nc()
    nc = _CACHE["nc"]

    in_maps = make_in_maps(u_vecs, W)
    res = bass_utils.run_bass_kernel_spmd(nc, in_maps, core_ids=list(range(NCORES)))
    s3 = np.concatenate([r["out"] for r in res.results], axis=0)  # [B, KND] raw s
    out = _squash_np(s3.astype(np.float32))
    return out.reshape(B, NCAP, DCAP).astype(np.float32)


# revision 13
# speedup vs baseline: 1.6714x; 1.1359x over previous
"""Trainium2 Bass kernel for CapsNet dynamic routing (nn_Capsule_13692355740297).

Math (per batch element):
    u_hat[i, (n,d)] = u[i, :] @ W[:, (n,d)]            # never materialized
    iter1: c uniform 1/10  -> s1 = 0.1 * (sum_i u_i) W  (c-independent => host)
    iter k: b[i, n] = u_i . P_n   with P_n = W_n o_n    # lhsT=UT tile (FWL)
            c = softmax_n(b)                            # free-dim softmax, [i,n]
            R^T[d, n] = sum_i u_i c[i, n]               # lhsT=U tile (FWL), rhs=cc
            s[n, :] = R_n @ W_n                         # mask-mult + ones matmul
            o = squash(s)                               # iter2 on-chip, iter3 host
Sharding: data-parallel over batch, 8 batch elements per core, no collectives.

Perf notes (HW-measured):
  - (128-col FWL LDWEIGHTS + 10-col MATMUL) pairs pipeline at ~27 ns;
    big moving streams serialize at ~128 ns/tile -> keep 10-col moving operands
    on both matmul flavors and feed the bulk data through the FWL weight path.
  - Scalar act-table reloads cost 1.5 us each and the stock chooser ping-pongs
    between per-func sets; we pin every scalar func (exp/ln/square/copy) to the
    combined natural_log_exp_and_others set via get_activation_tables.
    sqrt(q) = exp(0.5*ln(q)) keeps squash inside that one set.
  - 5-deep per-batch software pipeline so every cross-engine chain
    (softmax ~2us, squash+V3 ~2.5us) gets >= 1 slot (~3.5us) of slack
    before the tensor engine consumes its result.
"""

import numpy as np

B, I_FULL, DIN = 64, 4096, 128
NCAP, DCAP = 10, 16
KND = NCAP * DCAP  # 160
NCORES = 8
BC = B // NCORES  # 8 batch elements per core
NT = I_FULL // 128  # 32 i-tiles per batch
EPS = 1e-7
FP8 = True
GAMMA = 32.0 if FP8 else 1.0
ACT_SET = "natural_log_exp_and_others"


def build_nc(bc=BC, nt=NT, fp8=FP8):
    import concourse.bacc as bacc
    import concourse.mybir as mybir
    from concourse.tile import TileContext

    fp32 = mybir.dt.float32
    bf16 = mybir.dt.bfloat16
    dtu = mybir.dt.float8e3 if fp8 else mybir.dt.bfloat16
    AX = mybir.AxisListType
    ALU = mybir.AluOpType
    ACTF = mybir.ActivationFunctionType

    # Pin exp/ln/square/copy/identity to the single combined act-table set so
    # the table is loaded once instead of ping-ponging (1.5us per reload on the
    # softmax critical path). Set ids stay positionally valid; walrus loads the
    # real combined set which does contain all of these funcs.
    mine = {ACTF.Exp, ACTF.Ln, ACTF.Square, ACTF.Copy, ACTF.Identity}
    orig_fn = bacc.get_activation_tables

    def patched_tables(arch):
        t = orig_fn(arch)
        for name, funcs in t.items():
            if name != ACT_SET:
                funcs.difference_update(mine)
        return t

    il = nt * 128  # I per batch

    nc = bacc.Bacc(trn_type="TRN2")
    ut_h = nc.dram_tensor("ut", [bc, 128, il], dtu, kind="ExternalInput")
    u16_h = nc.dram_tensor("u16", [128, bc * il], bf16, kind="ExternalInput")
    v2_h = nc.dram_tensor("v2", [128, bc * NCAP], dtu, kind="ExternalInput")
    w32_h = nc.dram_tensor("w32", [128, KND], fp32, kind="ExternalInput")
    wt_hi_h = nc.dram_tensor("wt_hi", [128, DIN], bf16, kind="ExternalInput")
    wt_lo_h = nc.dram_tensor("wt_lo", [32, DIN], bf16, kind="ExternalInput")
    m_hi_h = nc.dram_tensor("m_hi", [128, NCAP], fp32, kind="ExternalInput")
    m_lo_h = nc.dram_tensor("m_lo", [32, NCAP], fp32, kind="ExternalInput")
    identf_h = nc.dram_tensor("identf", [32, 32], fp32, kind="ExternalInput")
    ones_h = nc.dram_tensor("ones", [128, 1], bf16, kind="ExternalInput")
    out_h = nc.dram_tensor("out", [bc, KND], fp32, kind="ExternalOutput")

    with TileContext(nc) as tc:
        with (
            tc.tile_pool(name="big", bufs=1) as big,
            tc.tile_pool(name="sb3", bufs=3) as sb3,
            tc.tile_pool(name="sb4", bufs=4) as sb4,
            tc.tile_pool(name="psB", bufs=2, space="PSUM") as psB,
            tc.tile_pool(name="psR", bufs=2, space="PSUM") as psR,
            tc.tile_pool(name="psS", bufs=2, space="PSUM") as psS,
            tc.tile_pool(name="psT", bufs=2, space="PSUM") as psT,
        ):
            # ---------- persistent SBUF ----------
            UT = big.tile([128, bc * il], dtu, name="UT_sb")   # [d, (b,i)]
            U16 = big.tile([128, bc * il], bf16, name="U16_sb")  # [p, (b,j,d)]
            V2 = big.tile([128, bc * NCAP], dtu, name="V2_sb")
            w32 = big.tile([128, KND], fp32, name="w32_sb")
            wt_hi = big.tile([128, DIN], bf16, name="wt_hi_sb")
            wt_lo = big.tile([32, DIN], bf16, name="wt_lo_sb")
            m_hi = big.tile([128, NCAP], fp32, name="m_hi_sb")
            m_lo = big.tile([32, NCAP], fp32, name="m_lo_sb")
            identf = big.tile([32, 32], fp32, name="identf_sb")
            onesb = big.tile([128, 1], bf16, name="ones_sb")

            nc.sync.dma_start(out=V2[:, :], in_=v2_h.ap())
            nc.sync.dma_start(out=w32[:, :], in_=w32_h.ap())
            nc.sync.dma_start(out=wt_hi[:, :], in_=wt_hi_h.ap())
            nc.sync.dma_start(out=wt_lo[:, :], in_=wt_lo_h.ap())
            nc.sync.dma_start(out=m_hi[:, :], in_=m_hi_h.ap())
            nc.sync.dma_start(out=m_lo[:, :], in_=m_lo_h.ap())
            nc.sync.dma_start(out=identf[:, :], in_=identf_h.ap())
            nc.sync.dma_start(out=onesb[:, :], in_=ones_h.ap())

            UTv = UT[:, :].rearrange("p (b i) -> p b i", b=bc, i=il)
            U16v = U16[:, :].rearrange("p (b j d) -> p b j d", b=bc, j=nt, d=128)
            Wv = w32[:, :].rearrange("p (n d) -> p n d", n=NCAP)

            # ---------- bulk load, batch-pipelined; UT and U16 on different queues
            for b in range(bc):
                nc.gpsimd.dma_start(out=UTv[:, b, :], in_=ut_h.ap()[b])
                half = il // 2
                nc.gpsimd.dma_start(
                    out=U16[:, b * il : b * il + half],
                    in_=u16_h.ap()[:, b * il : b * il + half],
                )
                nc.gpsimd.dma_start(
                    out=U16[:, b * il + half : (b + 1) * il],
                    in_=u16_h.ap()[:, b * il + half : (b + 1) * il],
                )

            cc_t, sp_t, ob_t, V3_t, prod_t = {}, {}, {}, {}, {}

            def logits(it, b, Vb):
                """32 (FWL-LDW + 10-col MM) pairs + exp + softmax -> cc (bf16)."""
                btp = psB.tile([128, nt * NCAP], fp32, name=f"btp{it}_{b}", tag="btp")
                for j in range(nt):
                    nc.tensor.matmul(
                        btp[:, NCAP * j : NCAP * (j + 1)],
                        UTv[:, b, 128 * j : 128 * (j + 1)],
                        Vb,
                    )
                eb = sb3.tile([128, nt * NCAP], fp32, name=f"eb{it}_{b}", tag="eb")
                nc.scalar.activation(eb[:, :], btp[:, :], ACTF.Exp, scale=1.0 / GAMMA)
                ebv = eb[:, :].rearrange("p (j n) -> p j n", j=nt)
                Z = sb3.tile([128, nt], fp32, name=f"Z{it}_{b}", tag="Z")
                nc.vector.reduce_sum(out=Z[:, :], in_=ebv, axis=AX.X, op=ALU.add)
                rZ = sb3.tile([128, nt], fp32, name=f"rZ{it}_{b}", tag="rZ")
                nc.vector.reciprocal(out=rZ[:, :], in_=Z[:, :])
                cc = sb4.tile([128, nt * NCAP], bf16, name=f"cc{it}_{b}", tag="cc")
                nc.vector.tensor_tensor(
                    out=cc[:, :].rearrange("p (j n) -> p j n", j=nt),
                    in0=ebv,
                    in1=rZ[:, :].unsqueeze(2).broadcast_to([128, nt, NCAP]),
                    op=ALU.mult,
                )
                cc_t[(it, b)] = cc

            def r_chain(it, b):
                """R^T = sum_j U_j @ cc_j -> Rp [128 d, 10 n]; prod = (R*W) bf16."""
                cc = cc_t.pop((it, b))
                Rp = psR.tile([128, NCAP], fp32, name=f"Rp{it}_{b}", tag="Rp")
                for j in range(nt):
                    nc.tensor.matmul(
                        Rp[:, :],
                        U16v[:, b, j],
                        cc[:, NCAP * j : NCAP * (j + 1)],
                        start=(j == 0),
                        stop=(j == nt - 1),
                    )
                prod = sb3.tile([128, KND], bf16, name=f"prod{it}_{b}", tag="prod")
                nc.vector.tensor_tensor(
                    out=prod[:, :].rearrange("p (n d) -> p n d", n=NCAP),
                    in0=Rp[:, :].unsqueeze(2).broadcast_to([128, NCAP, DCAP]),
                    in1=Wv,
                    op=ALU.mult,
                )
                prod_t[(it, b)] = prod

            def ones_mm(it, b):
                sp = psS.tile([1, KND], fp32, name=f"sp{it}_{b}", tag="sp")
                nc.tensor.matmul(sp[:, :], onesb[:, :], prod_t.pop((it, b))[:, :])
                sp_t[(it, b)] = sp

            def squash2(b):
                """squash on [1, KND]; sqrt via exp(0.5*ln) (same act set)."""
                sp = sp_t.pop((2, b))
                sq = sb3.tile([1, KND], fp32, name=f"sq{b}", tag="sq")
                nc.scalar.square(out=sq[:, :], in_=sp[:, :])
                q = sb3.tile([1, NCAP], fp32, name=f"q{b}", tag="q")
                nc.vector.reduce_sum(
                    out=q[:, :],
                    in_=sq[:, :].rearrange("p (n d) -> p n d", n=NCAP),
                    axis=AX.X, op=ALU.add,
                )
                qe = sb3.tile([1, NCAP], fp32, name=f"qe{b}", tag="qe")
                nc.vector.tensor_scalar_add(qe[:, :], q[:, :], EPS)
                lq = sb3.tile([1, NCAP], fp32, name=f"lq{b}", tag="lq")
                nc.scalar.activation(lq[:, :], qe[:, :], ACTF.Ln)
                rt = sb3.tile([1, NCAP], fp32, name=f"rt{b}", tag="rt")
                nc.scalar.activation(rt[:, :], lq[:, :], ACTF.Exp, scale=0.5)
                den = sb3.tile([1, NCAP], fp32, name=f"den{b}", tag="den")
                nc.vector.tensor_scalar_add(den[:, :], qe[:, :], 1.0)
                rden = sb3.tile([1, NCAP], fp32, name=f"rden{b}", tag="rden")
                nc.vector.reciprocal(out=rden[:, :], in_=den[:, :])
                coef = sb3.tile([1, NCAP], fp32, name=f"coef{b}", tag="coef")
                nc.vector.tensor_tensor(
                    out=coef[:, :], in0=rt[:, :], in1=rden[:, :], op=ALU.mult
                )
                ob = sb3.tile([1, KND], fp32, name=f"ob{b}", tag="ob")
                nc.vector.tensor_tensor(
                    out=ob[:, :].rearrange("p (n d) -> p n d", n=NCAP),
                    in0=sp[:, :].rearrange("p (n d) -> p n d", n=NCAP),
                    in1=coef[:, :].unsqueeze(2).broadcast_to([1, NCAP, DCAP]),
                    op=ALU.mult,
                )
                ob_t[b] = ob

            def build_V3(b):
                """V3(b) = gamma * W_n @ o_n from ob [1, KND] f32; masks carry gamma."""
                ob = ob_t.pop(b)
                oth_p = psT.tile([128, 1], fp32, name=f"oth{b}", tag="tp")
                otl_p = psT.tile([32, 1], fp32, name=f"otl{b}", tag="tp")
                nc.tensor.transpose(oth_p[:, :], ob[:, 0:128], identf[:1, :1])
                nc.tensor.transpose(otl_p[:, :], ob[:, 128:KND], identf[:1, :1])
                oeh = sb3.tile([128, NCAP], bf16, name=f"oeh{b}", tag="oeh")
                oel = sb3.tile([32, NCAP], bf16, name=f"oel{b}", tag="oel")
                nc.vector.tensor_tensor(
                    out=oeh[:, :],
                    in0=oth_p[:, :].broadcast_to([128, NCAP]),
                    in1=m_hi[:, :],
                    op=ALU.mult,
                )
                nc.vector.tensor_tensor(
                    out=oel[:, :],
                    in0=otl_p[:, :].broadcast_to([32, NCAP]),
                    in1=m_lo[:, :],
                    op=ALU.mult,
                )
                vp = psT.tile([128, NCAP], fp32, name=f"vp{b}", tag="tp")
                nc.tensor.matmul(vp[:, :], wt_hi[:, :], oeh[:, :], start=True, stop=False)
                nc.tensor.matmul(vp[:, :], wt_lo[:, :], oel[:, :], start=False, stop=True)
                V3 = sb3.tile([128, NCAP], dtu, name=f"V3_{b}", tag="V3")
                nc.scalar.copy(out=V3[:, :], in_=vp[:, :])
                V3_t[b] = V3

            def finish3(b):
                sp = sp_t.pop((3, b))
                o3 = sb3.tile([1, KND], fp32, name=f"o3_{b}", tag="o3")
                nc.vector.tensor_copy(out=o3[:, :], in_=sp[:, :])
                nc.sync.dma_start(out=out_h.ap()[b], in_=o3[:, :])

            # ---------- 6-deep per-batch software pipeline ----------
            # stage -> slot: L2(b)@b, R2(b)@b+1, ones2+squash2(b)@b+2,
            # V3mm(b)@b+2(end), L3(b)@b+3, R3(b)@b+4, ones3+out(b)@b+5
            for t in range(bc + 6):
                if t < bc:
                    logits(2, t, V2[:, NCAP * t : NCAP * (t + 1)])     # g1
                if 2 <= t <= bc + 1:
                    ones_mm(2, t - 2)                                   # g2
                    squash2(t - 2)                # chain overlaps g3-g6
                if 3 <= t <= bc + 2:
                    logits(3, t - 3, V3_t.pop(t - 3)[:, :])             # g3
                if 1 <= t <= bc:
                    r_chain(2, t - 1)                                   # g4
                if 5 <= t <= bc + 4:
                    ones_mm(3, t - 5)                                   # g5
                    finish3(t - 5)
                if 4 <= t <= bc + 3:
                    r_chain(3, t - 4)                                   # g6
                if 2 <= t <= bc + 1:
                    build_V3(t - 2)                                     # g7

    import concourse.bacc as bacc_mod
    bacc_mod.get_activation_tables = patched_tables
    try:
        nc.compile()
    finally:
        bacc_mod.get_activation_tables = orig_fn
    return nc


def _squash_np(s):
    sq = (s * s).reshape(s.shape[0], NCAP, DCAP).sum(-1, keepdims=True) + EPS
    coef = np.sqrt(sq) / (1.0 + sq)
    return (coef * s.reshape(s.shape[0], NCAP, DCAP)).reshape(s.shape)


def make_in_maps(u_vecs, W, fp8=FP8):
    import ml_dtypes

    bf = ml_dtypes.bfloat16
    f8 = ml_dtypes.float8_e3m4 if fp8 else bf

    u = np.asarray(u_vecs, dtype=np.float32)
    W = np.asarray(W, dtype=np.float32)

    # host iter-1 (c uniform): o1 = squash(0.1 * (sum_i u_i) @ W), V2 = g*W_n@o1_n
    r0 = u.sum(axis=1)                      # [B, 128]
    o1 = _squash_np(0.1 * (r0 @ W))         # [B, 160]
    V2full = np.einsum(
        "dnk,bnk->bdn",
        W.reshape(DIN, NCAP, DCAP),
        o1.reshape(-1, NCAP, DCAP),
        optimize=True,
    )                                        # [B, 128, 10]

    mask = np.zeros((KND, NCAP), dtype=np.float32)
    for k in range(KND):
        mask[k, k // DCAP] = GAMMA
    WT = W.T.copy()
    consts = {
        "w32": W,
        "wt_hi": WT[:128].astype(bf),
        "wt_lo": WT[128:].astype(bf),
        "m_hi": mask[:128],
        "m_lo": mask[128:],
        "identf": np.eye(32, dtype=np.float32),
        "ones": np.ones((128, 1), dtype=np.float32).astype(bf),
    }

    in_maps = []
    for c in range(NCORES):
        sl = u[c * BC : (c + 1) * BC]       # [8, 4096, 128]
        ut = np.ascontiguousarray(sl.transpose(0, 2, 1)).astype(f8)  # [8,128,4096]
        u16 = np.ascontiguousarray(
            sl.reshape(BC, NT, 128, 128).transpose(2, 0, 1, 3)
        ).reshape(128, BC * I_FULL).astype(bf)
        v2 = np.ascontiguousarray(
            V2full[c * BC : (c + 1) * BC].transpose(1, 0, 2)
        ).reshape(128, BC * NCAP)
        m = {"ut": ut, "u16": u16, "v2": (GAMMA * v2).astype(f8)}
        m.update(consts)
        in_maps.append(m)
    return in_maps


_CACHE = {}


def kernel(u_vecs, W):
    from concourse import bass_utils

    if "nc" not in _CACHE:
        _CACHE["nc"] = build_nc()
    nc = _CACHE["nc"]

    in_maps = make_in_maps(u_vecs, W)
    res = bass_utils.run_bass_kernel_spmd(nc, in_maps, core_ids=list(range(NCORES)))
    s3 = np.concatenate([r["out"] for r in res.results], axis=0)  # [B, KND] raw s
    out = _squash_np(s3.astype(np.float32))
    return out.reshape(B, NCAP, DCAP).astype(np.float32)


# revision 14
# speedup vs baseline: 1.6932x; 1.0131x over previous
"""Trainium2 Bass kernel for CapsNet dynamic routing (nn_Capsule_13692355740297).

Math (per batch element):
    u_hat[i, (n,d)] = u[i, :] @ W[:, (n,d)]            # never materialized
    iter1: c uniform 1/10  -> s1 = 0.1 * (sum_i u_i) W  (c-independent => host)
    iter k: b[i, n] = u_i . P_n   with P_n = W_n o_n    # lhsT=UT tile (FWL)
            c = softmax_n(b)                            # free-dim softmax, [i,n]
            R^T[d, n] = sum_i u_i c[i, n]               # lhsT=U tile (FWL), rhs=cc
            s[n, :] = R_n @ W_n                         # mask-mult + ones matmul
            o = squash(s)                               # iter2 on-chip, iter3 host
Sharding: data-parallel over batch, 8 batch elements per core, no collectives.

Perf notes (HW-measured):
  - (128-col FWL LDWEIGHTS + 10-col MATMUL) pairs pipeline at ~27 ns;
    big moving streams serialize at ~128 ns/tile -> keep 10-col moving operands
    on both matmul flavors and feed the bulk data through the FWL weight path.
  - Scalar act-table reloads cost 1.5 us each and the stock chooser ping-pongs
    between per-func sets; we pin every scalar func (exp/ln/square/copy) to the
    combined natural_log_exp_and_others set via get_activation_tables.
    sqrt(q) = exp(0.5*ln(q)) keeps squash inside that one set.
  - 5-deep per-batch software pipeline so every cross-engine chain
    (softmax ~2us, squash+V3 ~2.5us) gets >= 1 slot (~3.5us) of slack
    before the tensor engine consumes its result.
"""

import numpy as np

B, I_FULL, DIN = 64, 4096, 128
NCAP, DCAP = 10, 16
KND = NCAP * DCAP  # 160
NCORES = 8
BC = B // NCORES  # 8 batch elements per core
NT = I_FULL // 128  # 32 i-tiles per batch
EPS = 1e-7
FP8 = True
GAMMA = 32.0 if FP8 else 1.0
ACT_SET = "natural_log_exp_and_others"


def build_nc(bc=BC, nt=NT, fp8=FP8):
    import concourse.bacc as bacc
    import concourse.mybir as mybir
    from concourse.tile import TileContext

    fp32 = mybir.dt.float32
    bf16 = mybir.dt.bfloat16
    dtu = mybir.dt.float8e3 if fp8 else mybir.dt.bfloat16
    AX = mybir.AxisListType
    ALU = mybir.AluOpType
    ACTF = mybir.ActivationFunctionType

    # Pin exp/ln/square/copy/identity to the single combined act-table set so
    # the table is loaded once instead of ping-ponging (1.5us per reload on the
    # softmax critical path). Set ids stay positionally valid; walrus loads the
    # real combined set which does contain all of these funcs.
    mine = {ACTF.Exp, ACTF.Ln, ACTF.Square, ACTF.Copy, ACTF.Identity}
    orig_fn = bacc.get_activation_tables

    def patched_tables(arch):
        t = orig_fn(arch)
        for name, funcs in t.items():
            if name != ACT_SET:
                funcs.difference_update(mine)
        return t

    il = nt * 128  # I per batch

    nc = bacc.Bacc(trn_type="TRN2")
    ut_h = nc.dram_tensor("ut", [bc, 128, il], dtu, kind="ExternalInput")
    u16_h = nc.dram_tensor("u16", [128, bc * il], bf16, kind="ExternalInput")
    v2_h = nc.dram_tensor("v2", [128, bc * NCAP], dtu, kind="ExternalInput")
    w32_h = nc.dram_tensor("w32", [128, KND], fp32, kind="ExternalInput")
    wt_hi_h = nc.dram_tensor("wt_hi", [128, DIN], bf16, kind="ExternalInput")
    wt_lo_h = nc.dram_tensor("wt_lo", [32, DIN], bf16, kind="ExternalInput")
    m_hi_h = nc.dram_tensor("m_hi", [128, NCAP], fp32, kind="ExternalInput")
    m_lo_h = nc.dram_tensor("m_lo", [32, NCAP], fp32, kind="ExternalInput")
    identf_h = nc.dram_tensor("identf", [32, 32], fp32, kind="ExternalInput")
    ones_h = nc.dram_tensor("ones", [128, 1], bf16, kind="ExternalInput")
    out_h = nc.dram_tensor("out", [bc, KND], fp32, kind="ExternalOutput")

    with TileContext(nc) as tc:
        with (
            tc.tile_pool(name="big", bufs=1) as big,
            tc.tile_pool(name="sb3", bufs=3) as sb3,
            tc.tile_pool(name="sb4", bufs=4) as sb4,
            tc.tile_pool(name="psB", bufs=2, space="PSUM") as psB,
            tc.tile_pool(name="psR", bufs=2, space="PSUM") as psR,
            tc.tile_pool(name="psS", bufs=2, space="PSUM") as psS,
            tc.tile_pool(name="psT", bufs=2, space="PSUM") as psT,
        ):
            # ---------- persistent SBUF ----------
            UT = big.tile([128, bc * il], dtu, name="UT_sb")   # [d, (b,i)]
            U16 = big.tile([128, bc * il], bf16, name="U16_sb")  # [p, (b,j,d)]
            V2 = big.tile([128, bc * NCAP], dtu, name="V2_sb")
            w32 = big.tile([128, KND], fp32, name="w32_sb")
            wt_hi = big.tile([128, DIN], bf16, name="wt_hi_sb")
            wt_lo = big.tile([32, DIN], bf16, name="wt_lo_sb")
            m_hi = big.tile([128, NCAP], fp32, name="m_hi_sb")
            m_lo = big.tile([32, NCAP], fp32, name="m_lo_sb")
            identf = big.tile([32, 32], fp32, name="identf_sb")
            onesb = big.tile([128, 1], bf16, name="ones_sb")

            nc.sync.dma_start(out=V2[:, :], in_=v2_h.ap())
            nc.sync.dma_start(out=w32[:, :], in_=w32_h.ap())
            nc.sync.dma_start(out=onesb[:, :], in_=ones_h.ap())
            nc.scalar.dma_start(out=wt_hi[:, :], in_=wt_hi_h.ap())
            nc.scalar.dma_start(out=wt_lo[:, :], in_=wt_lo_h.ap())
            nc.scalar.dma_start(out=m_hi[:, :], in_=m_hi_h.ap())
            nc.scalar.dma_start(out=m_lo[:, :], in_=m_lo_h.ap())
            nc.scalar.dma_start(out=identf[:, :], in_=identf_h.ap())

            UTv = UT[:, :].rearrange("p (b i) -> p b i", b=bc, i=il)
            U16v = U16[:, :].rearrange("p (b j d) -> p b j d", b=bc, j=nt, d=128)
            Wv = w32[:, :].rearrange("p (n d) -> p n d", n=NCAP)

            # ---------- bulk load, batch-pipelined; UT and U16 on different queues
            for b in range(bc):
                nc.gpsimd.dma_start(out=UTv[:, b, :], in_=ut_h.ap()[b])
                nc.gpsimd.dma_start(
                    out=U16[:, b * il : (b + 1) * il],
                    in_=u16_h.ap()[:, b * il : (b + 1) * il],
                )

            cc_t, sp_t, ob_t, V3_t, prod_t = {}, {}, {}, {}, {}

            def logits(it, b, Vb):
                """32 (FWL-LDW + 10-col MM) pairs + exp + softmax -> cc (bf16)."""
                btp = psB.tile([128, nt * NCAP], fp32, name=f"btp{it}_{b}", tag="btp")
                for j in range(nt):
                    nc.tensor.matmul(
                        btp[:, NCAP * j : NCAP * (j + 1)],
                        UTv[:, b, 128 * j : 128 * (j + 1)],
                        Vb,
                    )
                eb = sb3.tile([128, nt * NCAP], fp32, name=f"eb{it}_{b}", tag="eb")
                nc.scalar.activation(eb[:, :], btp[:, :], ACTF.Exp, scale=1.0 / GAMMA)
                ebv = eb[:, :].rearrange("p (j n) -> p j n", j=nt)
                Z = sb3.tile([128, nt], fp32, name=f"Z{it}_{b}", tag="Z")
                nc.vector.reduce_sum(out=Z[:, :], in_=ebv, axis=AX.X, op=ALU.add)
                rZ = sb3.tile([128, nt], fp32, name=f"rZ{it}_{b}", tag="rZ")
                nc.vector.reciprocal(out=rZ[:, :], in_=Z[:, :])
                cc = sb4.tile([128, nt * NCAP], bf16, name=f"cc{it}_{b}", tag="cc")
                nc.vector.tensor_tensor(
                    out=cc[:, :].rearrange("p (j n) -> p j n", j=nt),
                    in0=ebv,
                    in1=rZ[:, :].unsqueeze(2).broadcast_to([128, nt, NCAP]),
                    op=ALU.mult,
                )
                cc_t[(it, b)] = cc

            def r_chain(it, b):
                """R^T = sum_j U_j @ cc_j -> Rp [128 d, 10 n]; prod = (R*W) bf16."""
                cc = cc_t.pop((it, b))
                Rp = psR.tile([128, NCAP], fp32, name=f"Rp{it}_{b}", tag="Rp")
                for j in range(nt):
                    nc.tensor.matmul(
                        Rp[:, :],
                        U16v[:, b, j],
                        cc[:, NCAP * j : NCAP * (j + 1)],
                        start=(j == 0),
                        stop=(j == nt - 1),
                    )
                prod = sb3.tile([128, KND], bf16, name=f"prod{it}_{b}", tag="prod")
                nc.vector.tensor_tensor(
                    out=prod[:, :].rearrange("p (n d) -> p n d", n=NCAP),
                    in0=Rp[:, :].unsqueeze(2).broadcast_to([128, NCAP, DCAP]),
                    in1=Wv,
                    op=ALU.mult,
                )
                prod_t[(it, b)] = prod

            def ones_mm(it, b):
                sp = psS.tile([1, KND], fp32, name=f"sp{it}_{b}", tag="sp")
                nc.tensor.matmul(sp[:, :], onesb[:, :], prod_t.pop((it, b))[:, :])
                sp_t[(it, b)] = sp

            def squash2(b):
                """squash on [1, KND]; sqrt via exp(0.5*ln) (same act set)."""
                sp = sp_t.pop((2, b))
                sq = sb3.tile([1, KND], fp32, name=f"sq{b}", tag="sq")
                nc.scalar.square(out=sq[:, :], in_=sp[:, :])
                q = sb3.tile([1, NCAP], fp32, name=f"q{b}", tag="q")
                nc.vector.reduce_sum(
                    out=q[:, :],
                    in_=sq[:, :].rearrange("p (n d) -> p n d", n=NCAP),
                    axis=AX.X, op=ALU.add,
                )
                qe = sb3.tile([1, NCAP], fp32, name=f"qe{b}", tag="qe")
                nc.vector.tensor_scalar_add(qe[:, :], q[:, :], EPS)
                lq = sb3.tile([1, NCAP], fp32, name=f"lq{b}", tag="lq")
                nc.scalar.activation(lq[:, :], qe[:, :], ACTF.Ln)
                rt = sb3.tile([1, NCAP], fp32, name=f"rt{b}", tag="rt")
                nc.scalar.activation(rt[:, :], lq[:, :], ACTF.Exp, scale=0.5)
                den = sb3.tile([1, NCAP], fp32, name=f"den{b}", tag="den")
                nc.vector.tensor_scalar_add(den[:, :], qe[:, :], 1.0)
                rden = sb3.tile([1, NCAP], fp32, name=f"rden{b}", tag="rden")
                nc.vector.reciprocal(out=rden[:, :], in_=den[:, :])
                coef = sb3.tile([1, NCAP], fp32, name=f"coef{b}", tag="coef")
                nc.vector.tensor_tensor(
                    out=coef[:, :], in0=rt[:, :], in1=rden[:, :], op=ALU.mult
                )
                ob = sb3.tile([1, KND], fp32, name=f"ob{b}", tag="ob")
                nc.vector.tensor_tensor(
                    out=ob[:, :].rearrange("p (n d) -> p n d", n=NCAP),
                    in0=sp[:, :].rearrange("p (n d) -> p n d", n=NCAP),
                    in1=coef[:, :].unsqueeze(2).broadcast_to([1, NCAP, DCAP]),
                    op=ALU.mult,
                )
                ob_t[b] = ob

            def build_V3(b):
                """V3(b) = gamma * W_n @ o_n from ob [1, KND] f32; masks carry gamma."""
                ob = ob_t.pop(b)
                oth_p = psT.tile([128, 1], fp32, name=f"oth{b}", tag="tp")
                otl_p = psT.tile([32, 1], fp32, name=f"otl{b}", tag="tp")
                nc.tensor.transpose(oth_p[:, :], ob[:, 0:128], identf[:1, :1])
                nc.tensor.transpose(otl_p[:, :], ob[:, 128:KND], identf[:1, :1])
                oeh = sb3.tile([128, NCAP], bf16, name=f"oeh{b}", tag="oeh")
                oel = sb3.tile([32, NCAP], bf16, name=f"oel{b}", tag="oel")
                nc.vector.tensor_tensor(
                    out=oeh[:, :],
                    in0=oth_p[:, :].broadcast_to([128, NCAP]),
                    in1=m_hi[:, :],
                    op=ALU.mult,
                )
                nc.vector.tensor_tensor(
                    out=oel[:, :],
                    in0=otl_p[:, :].broadcast_to([32, NCAP]),
                    in1=m_lo[:, :],
                    op=ALU.mult,
                )
                vp = psT.tile([128, NCAP], fp32, name=f"vp{b}", tag="tp")
                nc.tensor.matmul(vp[:, :], wt_hi[:, :], oeh[:, :], start=True, stop=False)
                nc.tensor.matmul(vp[:, :], wt_lo[:, :], oel[:, :], start=False, stop=True)
                V3 = sb3.tile([128, NCAP], dtu, name=f"V3_{b}", tag="V3")
                nc.scalar.copy(out=V3[:, :], in_=vp[:, :])
                V3_t[b] = V3

            def finish3(b):
                sp = sp_t.pop((3, b))
                o3 = sb3.tile([1, KND], fp32, name=f"o3_{b}", tag="o3")
                nc.scalar.copy(out=o3[:, :], in_=sp[:, :])
                nc.sync.dma_start(out=out_h.ap()[b], in_=o3[:, :])

            # ---------- 6-deep per-batch software pipeline ----------
            # stage -> slot: L2(b)@b, R2(b)@b+1, ones2+squash2(b)@b+2,
            # V3mm(b)@b+2(end), L3(b)@b+3, R3(b)@b+4, ones3+out(b)@b+5
            for t in range(bc + 6):
                if 3 <= t <= bc + 2:
                    build_V3(t - 3)                                     # g0
                if t < bc:
                    logits(2, t, V2[:, NCAP * t : NCAP * (t + 1)])     # g1
                if 2 <= t <= bc + 1:
                    ones_mm(2, t - 2)                                   # g2
                    squash2(t - 2)                # chain overlaps g3-g6
                if 3 <= t <= bc + 2:
                    logits(3, t - 3, V3_t.pop(t - 3)[:, :])             # g3
                if 1 <= t <= bc:
                    r_chain(2, t - 1)                                   # g4
                if 5 <= t <= bc + 4:
                    ones_mm(3, t - 5)                                   # g5
                    finish3(t - 5)
                if 4 <= t <= bc + 3:
                    r_chain(3, t - 4)                                   # g6

    import concourse.bacc as bacc_mod
    bacc_mod.get_activation_tables = patched_tables
    try:
        nc.compile()
    finally:
        bacc_mod.get_activation_tables = orig_fn
    return nc


def _squash_np(s):
    sq = (s * s).reshape(s.shape[0], NCAP, DCAP).sum(-1, keepdims=True) + EPS
    coef = np.sqrt(sq) / (1.0 + sq)
    return (coef * s.reshape(s.shape[0], NCAP, DCAP)).reshape(s.shape)


def make_in_maps(u_vecs, W, fp8=FP8):
    import ml_dtypes

    bf = ml_dtypes.bfloat16
    f8 = ml_dtypes.float8_e3m4 if fp8 else bf

    u = np.asarray(u_vecs, dtype=np.float32)
    W = np.asarray(W, dtype=np.float32)

    # host iter-1 (c uniform): o1 = squash(0.1 * (sum_i u_i) @ W), V2 = g*W_n@o1_n
    r0 = u.sum(axis=1)                      # [B, 128]
    o1 = _squash_np(0.1 * (r0 @ W))         # [B, 160]
    V2full = np.einsum(
        "dnk,bnk->bdn",
        W.reshape(DIN, NCAP, DCAP),
        o1.reshape(-1, NCAP, DCAP),
        optimize=True,
    )                                        # [B, 128, 10]

    mask = np.zeros((KND, NCAP), dtype=np.float32)
    for k in range(KND):
        mask[k, k // DCAP] = GAMMA
    WT = W.T.copy()
    consts = {
        "w32": W,
        "wt_hi": WT[:128].astype(bf),
        "wt_lo": WT[128:].astype(bf),
        "m_hi": mask[:128],
        "m_lo": mask[128:],
        "identf": np.eye(32, dtype=np.float32),
        "ones": np.ones((128, 1), dtype=np.float32).astype(bf),
    }

    in_maps = []
    for c in range(NCORES):
        sl = u[c * BC : (c + 1) * BC]       # [8, 4096, 128]
        ut = np.ascontiguousarray(sl.transpose(0, 2, 1)).astype(f8)  # [8,128,4096]
        u16 = np.ascontiguousarray(
            sl.reshape(BC, NT, 128, 128).transpose(2, 0, 1, 3)
        ).reshape(128, BC * I_FULL).astype(bf)
        v2 = np.ascontiguousarray(
            V2full[c * BC : (c + 1) * BC].transpose(1, 0, 2)
        ).reshape(128, BC * NCAP)
        m = {"ut": ut, "u16": u16, "v2": (GAMMA * v2).astype(f8)}
        m.update(consts)
        in_maps.append(m)
    return in_maps


_CACHE = {}


def kernel(u_vecs, W):
    from concourse import bass_utils

    if "nc" not in _CACHE:
        _CACHE["nc"] = build_nc()
    nc = _CACHE["nc"]

    in_maps = make_in_maps(u_vecs, W)
    res = bass_utils.run_bass_kernel_spmd(nc, in_maps, core_ids=list(range(NCORES)))
    s3 = np.concatenate([r["out"] for r in res.results], axis=0)  # [B, KND] raw s
    out = _squash_np(s3.astype(np.float32))
    return out.reshape(B, NCAP, DCAP).astype(np.float32)
